# revision 3
# baseline (speedup 1.0000x reference)
"""Cross-attention reducer kernel for Trainium2, 8 NeuronCores (SPMD).

Problem (full shapes):
    token_input    [T=8192, L=4096]
    learned_queries[V=4096, I=512]
    w_q [I, I], w_k [L, I], w_v [L, I], w_out [I, L]

    q = learned_queries @ w_q;  k = token_input @ w_k;  v = token_input @ w_v
    per head h (H=8, D=64): attn = softmax(q_h k_h^T / sqrt(D)); out_h = attn @ v_h
    out = concat_h(out_h) @ w_out      -> [V, L]

Sharding: queries (V) are sharded 8 ways; the K/V projections are
sequence-parallel (each core projects its T/8 token shard) followed by an
AllGather of k^T and v, after which every core runs attention for all 8 heads
over its own 512 queries and the full gathered T, then applies the output
projection for its V-shard. Everything is computed transposed
(final^T = w_out^T-contraction) so every matmul contracts on the partition
dimension with no large transposes anywhere:

    q^T  [I, Vs]  = w_q (lhsT)  x lq^T (rhs)
    k^T  [I, t]   = w_k (lhsT)  x tok^T (rhs)        (gathered)
    v^T  [I, t]   = w_v (lhsT)  x tok^T (rhs), then 128x128 PE-transposes
                    to v [t, I] before the gather
    s^T  [t, Vs]  = k_h^T (lhsT) x q_h^T (rhs)       (t-tiles of 128)
    p^T           = exp(s^T / 8)                      (no max-subtraction:
                    scores are O(3), exp can't overflow; identical math)
    u^T  [D+1,Vs] = [v_h | 1] (lhsT) x p^T (rhs)     (row D = softmax denom)
    a^T  [D, Vs]  = u^T * (1/denom broadcast via PE outer product)
    out^T[L, Vs]  = w_out (lhsT) x a^T (rhs)

All inputs are cast to bf16 on the host (input-rounding contributes ~3e-3
max-rel error, tolerance 2e-2); all matmuls run bf16 at 1 cycle/col.

Overlap structure (the point of this version):
  - token shard loaded ONCE into SBUF (bf16, 64KB/partition), reused by both
    K and V projections.
  - AllGather-k is kicked immediately after the K projection, so it overlaps
    the V projection; AllGather-v overlaps the q projection and the head-0
    score pipeline (which runs ~8 psum-groups ahead of the first attn@v).
  - collective outputs are addr_space="Shared" (direct peer-HBM writes).
  - collectives + gather-dependent loads (vh_all, w_out prefetch, aT stores)
    issue on gpsimd; all other loads on sync, so a blocking collective can
    never stall the compute-feed queues.
"""

import os

import numpy as np
import ml_dtypes

import concourse.bacc as bacc
import concourse.tile as tile
import concourse.mybir as mybir
from concourse.bass_utils import run_bass_kernel_spmd

F32 = mybir.dt.float32
BF16 = mybir.dt.bfloat16
EXP = mybir.ActivationFunctionType.Exp
EQ = mybir.AluOpType.is_equal

N_CORES = 8
T, L, V, INNER = 8192, 4096, 4096, 512
H, D = 8, 64
TS = T // N_CORES      # 1024  t-shard per core
QS = V // N_CORES      # 512   query shard per core
SCALE = D ** -0.5      # 0.125

NT = T // 128          # 64 gathered t-tiles per head
GRP = 3                # t-tiles per exp batch (3 psum banks)


def build_program():
    nc = bacc.Bacc(
        "TRN2", target_bir_lowering=False, debug=False, num_devices=N_CORES
    )

    tok_T = nc.dram_tensor("tok_T", [L, TS], BF16, kind="ExternalInput").ap()
    lq_T = nc.dram_tensor("lq_T", [INNER, QS], BF16, kind="ExternalInput").ap()
    w_q = nc.dram_tensor("w_q", [INNER, INNER], BF16, kind="ExternalInput").ap()
    w_k = nc.dram_tensor("w_k", [L, INNER], BF16, kind="ExternalInput").ap()
    w_v = nc.dram_tensor("w_v", [L, INNER], BF16, kind="ExternalInput").ap()
    w_out = nc.dram_tensor("w_out", [INNER, L], BF16, kind="ExternalInput").ap()
    outT = nc.dram_tensor("outT", [L, QS], F32, kind="ExternalOutput").ap()

    # rearranged DRAM views (partition-major for SBUF loads)
    tok_v = tok_T.rearrange("(k p) t -> p k t", p=128)          # [128, 32, 1024]
    lq_v = lq_T.rearrange("(k p) q -> p k q", p=128)            # [128, 4, 512]
    w_q_v = w_q.rearrange("(k p) i -> p k i", p=128)            # [128, 4, 512]
    w_k_v = w_k.rearrange("(k p) i -> p k i", p=128)            # [128, 32, 512]
    w_v_v = w_v.rearrange("(k p) i -> p k i", p=128)            # [128, 32, 512]
    w_out_v = w_out.rearrange("(k p) l -> p k l", p=128)        # [128, 4, 4096]

    with tile.TileContext(nc) as tc:
        with (
            tc.tile_pool(name="persist", bufs=1) as persist,
            tc.tile_pool(name="dram", bufs=1, space="DRAM") as dram,
        ):
            # ---- persistent SBUF across phases ----
            qT_sb = persist.tile([64, H, QS], BF16, tag="qT")        # q^T per head
            aT_sb = persist.tile([128, 4, QS], BF16, tag="aT")       # attn out^T
            idn = persist.tile([128, 128], BF16, tag="idn")          # identity
            ones_64 = persist.tile([1, D], F32, tag="ones64")

            # collective bounce buffers
            gk_in = dram.tile([INNER, TS], BF16, tag="gk_in")
            gk_out = dram.tile(
                [N_CORES * INNER, TS], BF16, tag="gk_out", addr_space="Shared"
            )
            gv_in = dram.tile([TS, INNER], BF16, tag="gv_in")
            gv_out = dram.tile(
                [N_CORES * TS, INNER], BF16, tag="gv_out", addr_space="Shared"
            )
            gk_in_v = gk_in.rearrange("(m p) t -> p m t", p=128)    # [128, 4, 1024]
            gv_in_v = gv_in.rearrange("(j p) i -> p j i", p=128)    # [128, 8, 512]

            # identity matrix for PE transposes: idn[p, f] = (f == p)
            with tc.tile_pool(name="idpool", bufs=1) as idp:
                irow = idp.tile([128, 128], F32, tag="irow")
                icol = idp.tile([128, 1], F32, tag="icol")
                nc.gpsimd.iota(irow[:], pattern=[[1, 128]], base=0, channel_multiplier=0, allow_small_or_imprecise_dtypes=True)
                nc.gpsimd.iota(icol[:], pattern=[[0, 1]], base=0, channel_multiplier=1, allow_small_or_imprecise_dtypes=True)
                nc.vector.tensor_scalar(idn[:], irow[:], icol[:], None, EQ)
            nc.vector.memset(ones_64[:], 1.0)

            # ================= phase 1: projections =================
            with (
                tc.tile_pool(name="ptok", bufs=1) as ptok,
                tc.tile_pool(name="proj", bufs=2) as proj,
                tc.tile_pool(name="pps", bufs=2, space="PSUM") as pps,
            ):
                # token shard resident in SBUF, loaded once (2 chunked DMAs)
                tok_sb = ptok.tile([128, 32, TS], BF16, tag="tok")
                nc.sync.dma_start(tok_sb[:, :, 0:512], tok_v[:, :, 0:512])
                nc.sync.dma_start(tok_sb[:, :, 512:1024], tok_v[:, :, 512:1024])

                # --- k^T projection, bounced to gk_in per tile ---
                for m in range(4):
                    wcol = proj.tile([128, 32, 128], BF16, tag="wcol")
                    nc.sync.dma_start(wcol[:], w_k_v[:, :, m * 128:(m + 1) * 128])
                    for th in range(2):
                        ps = pps.tile([128, 512], F32, tag="pp")
                        for k in range(32):
                            nc.tensor.matmul(
                                ps[:], wcol[:, k, :], tok_sb[:, k, th * 512:(th + 1) * 512],
                                start=(k == 0), stop=(k == 31),
                            )
                        kstage = proj.tile([128, 512], BF16, tag="kstage")
                        nc.vector.tensor_copy(kstage[:], ps[:])
                        nc.sync.dma_start(
                            gk_in_v[:, m, th * 512:(th + 1) * 512], kstage[:]
                        )
                if os.environ.get("BASSK_NO_CC"):
                    nc.sync.dma_start(gk_out[0:INNER, :], gk_in[:])
                else:
                    nc.gpsimd.collective_compute(
                        "AllGather", mybir.AluOpType.bypass,
                        replica_groups=[list(range(N_CORES))],
                        ins=[gk_in.opt()], outs=[gk_out.opt()],
                    )

                # --- v^T projection + PE transpose to v [t, i], bounced ---
                for m in range(4):
                    wcol = proj.tile([128, 32, 128], BF16, tag="wcol")
                    nc.sync.dma_start(wcol[:], w_v_v[:, :, m * 128:(m + 1) * 128])
                    for th in range(2):
                        ps = pps.tile([128, 512], F32, tag="pp")
                        for k in range(32):
                            nc.tensor.matmul(
                                ps[:], wcol[:, k, :], tok_sb[:, k, th * 512:(th + 1) * 512],
                                start=(k == 0), stop=(k == 31),
                            )
                        vst = proj.tile([128, 512], BF16, tag="vst")
                        nc.vector.tensor_copy(vst[:], ps[:])
                        pt = pps.tile([128, 512], BF16, tag="pt")
                        for j in range(4):
                            nc.tensor.transpose(
                                pt[:, j * 128:(j + 1) * 128],
                                vst[:, j * 128:(j + 1) * 128],
                                idn[:],
                            )
                        # pt columns j hold v[t-chunk j of this half, i-block m]
                        vstage = proj.tile([128, 4, 128], BF16, tag="vstage")
                        nc.vector.tensor_copy(
                            vstage[:], pt[:].rearrange("p (j i) -> p j i", j=4)
                        )
                        nc.sync.dma_start(
                            gv_in_v[:, th * 4:(th + 1) * 4, m * 128:(m + 1) * 128],
                            vstage[:],
                        )
                if os.environ.get("BASSK_NO_CC"):
                    nc.sync.dma_start(gv_out[0:TS, :], gv_in[:])
                else:
                    nc.gpsimd.collective_compute(
                        "AllGather", mybir.AluOpType.bypass,
                        replica_groups=[list(range(N_CORES))],
                        ins=[gv_in.opt()], outs=[gv_out.opt()],
                    )

                # --- q^T projection ---
                wq_sb = proj.tile([128, 4, INNER], BF16, tag="wq", bufs=1)
                lq_sb = proj.tile([128, 4, QS], BF16, tag="lq", bufs=1)
                nc.sync.dma_start(wq_sb[:], w_q_v)
                nc.sync.dma_start(lq_sb[:], lq_v)
                for m in range(4):
                    ps = pps.tile([128, QS], F32, tag="pp")
                    for kk in range(4):
                        nc.tensor.matmul(
                            ps[:],
                            wq_sb[:, kk, m * 128:(m + 1) * 128],
                            lq_sb[:, kk, :],
                            start=(kk == 0), stop=(kk == 3),
                        )
                    qstage = proj.tile([128, QS], BF16, tag="qstage")
                    nc.vector.tensor_copy(qstage[:], ps[:])
                    # shift each head's 64 rows down to base partition 0
                    nc.sync.dma_start(qT_sb[:, 2 * m, :], qstage[0:64, :])
                    nc.sync.dma_start(qT_sb[:, 2 * m + 1, :], qstage[64:128, :])

            # ================= phase 2: attention =================
            gv_v = gv_out.rearrange("(x p) i -> p x i", p=128)       # [128, 64, 512]
            gk_head = gk_out.rearrange("(c p) t -> p c t", p=INNER)  # [512, 8, 1024]
            groups = [list(range(s, min(s + GRP, NT))) for s in range(0, NT, GRP)]

            with (
                tc.tile_pool(name="attn", bufs=2) as attn,
                tc.tile_pool(name="attn3", bufs=8) as attn3,
                tc.tile_pool(name="aps", bufs=2, space="PSUM") as aps,
                tc.tile_pool(name="aps1", bufs=1, space="PSUM") as aps1,
            ):
                # all heads' V loaded once (gpsimd: queued behind the gather)
                vh_all = attn.tile([128, NT, INNER], BF16, tag="vh_all", bufs=1)
                nc.gpsimd.dma_start(vh_all[:], gv_v)
                # w_out prefetch (no gather dep, but gpsimd is free after)
                wo_all = attn.tile([128, 4, L], BF16, tag="wo_all", bufs=1)
                nc.gpsimd.dma_start(wo_all[:], w_out_v)

                for h in range(H):
                    kTh = attn.tile([64, N_CORES, TS], BF16, tag="kTh")
                    nc.sync.dma_start(kTh[:], gk_head[h * D:(h + 1) * D, :, :])
                    vh = attn.tile([128, NT, D + 1], BF16, tag="vh")
                    nc.vector.memset(vh[:, :, D], 1.0)
                    nc.vector.tensor_copy(
                        vh[:, :, 0:D], vh_all[:, :, h * D:(h + 1) * D]
                    )
                    qTh = qT_sb[:, h, :]

                    ps_o = aps1.tile([D + 1, QS], F32, tag="ps_o")
                    # score pipeline runs LAG groups ahead of attn@v so the
                    # first head tolerates a late AllGather-v
                    lag = 7 if h == 0 else 1
                    pending = []
                    for g in groups:
                        ps_s = aps.tile([128, GRP * QS], F32, tag="ps_s")
                        for jj, j in enumerate(g):
                            nc.tensor.matmul(
                                ps_s[:, jj * QS:(jj + 1) * QS],
                                kTh[:, j // (TS // 128), (j % (TS // 128)) * 128:
                                    (j % (TS // 128)) * 128 + 128],
                                qTh,
                                start=True, stop=True,
                            )
                        pT = attn3.tile([128, GRP * QS], BF16, tag="pT")
                        n = len(g) * QS
                        nc.scalar.activation(pT[:, 0:n], ps_s[:, 0:n], EXP, scale=SCALE)
                        pending.append((g, pT))
                        if len(pending) > lag:
                            pg, ppT = pending.pop(0)
                            for jj, j in enumerate(pg):
                                nc.tensor.matmul(
                                    ps_o[:], vh[:, j, :], ppT[:, jj * QS:(jj + 1) * QS],
                                    start=(j == 0), stop=(j == NT - 1),
                                    skip_group_check=True,
                                )
                    for pg, ppT in pending:
                        for jj, j in enumerate(pg):
                            nc.tensor.matmul(
                                ps_o[:], vh[:, j, :], ppT[:, jj * QS:(jj + 1) * QS],
                                start=(j == 0), stop=(j == NT - 1),
                                skip_group_check=True,
                            )

                    # normalize: a^T = u^T / denom  (denom broadcast via PE)
                    u_sb = attn.tile([D + 1, QS], F32, tag="u")
                    nc.vector.tensor_copy(u_sb[:], ps_o[:])
                    dn0 = attn.tile([1, QS], F32, tag="dn0")
                    nc.sync.dma_start(dn0[:], u_sb[D:D + 1, :])  # shift to partition 0
                    recip = attn.tile([1, QS], F32, tag="recip")
                    nc.vector.reciprocal(recip[:], dn0[:])
                    ps_r = aps.tile([D, QS], F32, tag="ps_s")  # borrow a ps_s slot
                    nc.tensor.matmul(ps_r[:], ones_64[:], recip[:], start=True, stop=True)
                    a_tmp = attn.tile([D, QS], BF16, tag="a_tmp")
                    nc.vector.tensor_mul(a_tmp[:], u_sb[0:D, :], ps_r[:])
                    nc.gpsimd.dma_start(
                        aT_sb[(h % 2) * 64:(h % 2) * 64 + 64, h // 2, :], a_tmp[:]
                    )

                # ============ phase 3: output projection ============
                if True:
                    for m in range(L // 128):
                        ps = aps.tile([128, QS], F32, tag="ps_s")
                        for kk in range(4):
                            nc.tensor.matmul(
                                ps[:], wo_all[:, kk, m * 128:(m + 1) * 128],
                                aT_sb[:, kk, :],
                                start=(kk == 0), stop=(kk == 3),
                            )
                        of = attn.tile([128, QS], F32, tag="of", bufs=3)
                        nc.vector.tensor_copy(of[:], ps[:])
                        nc.sync.dma_start(outT[m * 128:(m + 1) * 128, :], of[:])

    nc.compile()
    return nc


_COMPILED = None


def _get_compiled():
    global _COMPILED
    if _COMPILED is None:
        _COMPILED = build_program()
    return _COMPILED


def _bf(x):
    return np.ascontiguousarray(np.asarray(x, dtype=np.float32)).astype(
        ml_dtypes.bfloat16
    )


def make_in_maps(token_input, learned_queries, w_q, w_k, w_v, w_out):
    token_input = np.asarray(token_input, dtype=np.float32)
    learned_queries = np.asarray(learned_queries, dtype=np.float32)
    w_q_b, w_k_b, w_v_b, w_out_b = _bf(w_q), _bf(w_k), _bf(w_v), _bf(w_out)
    in_maps = []
    for c in range(N_CORES):
        in_maps.append({
            "tok_T": _bf(token_input[c * TS:(c + 1) * TS, :].T),
            "lq_T": _bf(learned_queries[c * QS:(c + 1) * QS, :].T),
            "w_q": w_q_b, "w_k": w_k_b, "w_v": w_v_b, "w_out": w_out_b,
        })
    return in_maps


def assemble(results):
    out = np.empty((V, L), dtype=np.float32)
    for c in range(N_CORES):
        out[c * QS:(c + 1) * QS, :] = results[c]["outT"].T
    return out


def kernel(token_input, learned_queries, w_q, w_k, w_v, w_out):
    nc = _get_compiled()
    in_maps = make_in_maps(token_input, learned_queries, w_q, w_k, w_v, w_out)
    res = run_bass_kernel_spmd(nc, in_maps, list(range(N_CORES)))
    return assemble(res.results)
